# revision 22
# baseline (speedup 1.0000x reference)
"""MultiHeadAttention (B=4, S=2048, D=1024, H=16, causal) on 8 TRN2 NeuronCores.

Sharding: tensor-parallel over heads across all 8 cores (2 heads/core, all 4
batches processed locally; identical SPMD control flow on every core). After
attention, an 8-core AllToAll per batch redistributes attention outputs so
each core runs the output projection for 1/8 of the (batch, seq) rows.

Per-core pipeline (all matmuls bf16 with f32 PSUM accumulation):
  - x arrives host-transposed as x^T [D, B*S] bf16; K^T/Q^T/V^T via
    w-stationary matmuls (each weight load serves a pair of 512-wide chunks),
    bias added on the DVE eviction. V^T is transposed to natural V by the DMA
    XBAR engine (no PE involvement) with a ones column per head so the PV
    matmul also produces the softmax denominator.
  - Scores are computed transposed ([k, q] = K @ Q^T) in 512-wide q-chunks.
    The two heads' score matmuls are issued as row-group tiles (K=64 each,
    rows 0-63 / 64-127) so they execute CONCURRENTLY in the PE array and land
    in the two PSUM banks of one [128, 1024] tile; a single wide exp on
    ScalarE covers both heads (1/sqrt(dk) scale folded in; no max subtraction
    needed: |scores| <~ 2.6). Causal mask = 0/1 triangular multiply on
    diagonal tiles only; fully-masked tiles are never computed.
  - PV: [V_h|1] stationary, exp chunks stream, accumulating po_h = [out^T;
    denom] [65, 512] in PSUM, one k-strip behind scores/exp so the in-order
    PE queue never head-of-line blocks on ScalarE.
  - po is already in the [head-dim, q] layout the AllToAll needs, so there
    are no output transposes at all: the denominator row is reciprocal'd,
    bounced through DRAM to broadcast it across 64 partitions, and a single
    DVE multiply writes the normalized bf16 attention output.
  - Output projection for batch b runs right after its AllToAll, interleaved
    into the next batch's attention (its DMAs ride the GpSimd queue, which
    already serializes behind the collective).
"""

import sys

if "/opt/trn_rl_repo" not in sys.path:
    sys.path.insert(0, "/opt/trn_rl_repo")

from contextlib import ExitStack
from itertools import chain

import ml_dtypes
import numpy as np

import concourse.bacc as bacc
import concourse.bass as bass
import concourse.mybir as mybir
import concourse.tile as tile
from concourse.bass_utils import run_bass_kernel_spmd
from concourse.masks import make_upper_triangular

N_CORES = 8
B = 4
S = 2048
D = 1024
H_TOT = 16
DK = 64
H_LOC = H_TOT // N_CORES  # 2 heads per core
HC = H_LOC * DK  # 128 head-cols per core
ST = S // 128  # 16 k-strips per batch
DC = D // 128  # 8 d_model chunks
NT = S // 512  # 4 q-chunks per batch
BQ = (B * S) // N_CORES  # 1024 (batch,seq) rows per core after AllToAll

F32 = mybir.dt.float32
BF16 = mybir.dt.bfloat16
BF16_NP = ml_dtypes.bfloat16


def _bcast(handle, rows, cols):
    """AP reading a [1, cols] DRAM tensor broadcast over `rows` partitions."""
    return bass.AP(tensor=handle, offset=0, ap=[[0, rows], [1, cols]])


def build_program():
    nc = bacc.Bacc("TRN2", target_bir_lowering=False, debug=False,
                   num_devices=N_CORES)

    xt = nc.declare_dram_parameter("xt", [D, B * S], BF16, isOutput=False)
    wq = nc.declare_dram_parameter("wq", [D, HC], BF16, isOutput=False)
    wk = nc.declare_dram_parameter("wk", [D, HC], BF16, isOutput=False)
    wv = nc.declare_dram_parameter("wv", [D, HC], BF16, isOutput=False)
    bq = nc.declare_dram_parameter("bq", [HC, 1], F32, isOutput=False)
    bk = nc.declare_dram_parameter("bk", [HC, 1], F32, isOutput=False)
    bv = nc.declare_dram_parameter("bv", [HC, 1], F32, isOutput=False)
    wo = nc.declare_dram_parameter("wo", [D, D], BF16, isOutput=False)
    bo = nc.declare_dram_parameter("bo", [1, D], F32, isOutput=False)
    out = nc.declare_dram_parameter("out", [BQ, D], F32, isOutput=True)

    with ExitStack() as ctx:
        tc = ctx.enter_context(tile.TileContext(nc))

        consts = ctx.enter_context(tc.tile_pool(name="consts", bufs=1))
        wpool = ctx.enter_context(tc.tile_pool(name="wpool", bufs=1))
        xtp = ctx.enter_context(tc.tile_pool(name="xtp", bufs=2))
        kqv = ctx.enter_context(tc.tile_pool(name="kqv", bufs=2))
        epool = ctx.enter_context(tc.tile_pool(name="epool", bufs=4))
        # normalize-chain pools are deep enough (4 chunks = one full batch)
        # to ride out the first-collective setup (~56us) blocking the GpSimd
        # DMA queue without backpressuring the PV/exp pipeline
        rpool = ctx.enter_context(tc.tile_pool(name="rpool", bufs=5))
        bpool = ctx.enter_context(tc.tile_pool(name="bpool", bufs=5))
        spool = ctx.enter_context(tc.tile_pool(name="spool", bufs=10))
        opool = ctx.enter_context(tc.tile_pool(name="opool", bufs=2))
        atp = ctx.enter_context(tc.tile_pool(name="atp", bufs=2))
        ps_s = ctx.enter_context(tc.tile_pool(name="ps_s", bufs=2, space="PSUM"))
        ps_po = ctx.enter_context(tc.tile_pool(name="ps_po", bufs=2, space="PSUM"))
        ps_pp = ctx.enter_context(tc.tile_pool(name="ps_pp", bufs=2, space="PSUM"))
        dram = ctx.enter_context(tc.tile_pool(name="dram", bufs=1, space="DRAM"))

        # tiny dummy exchange issued at kernel start: absorbs the one-time
        # first-collective setup (~40us) concurrently with the initial
        # DMA/projection phase instead of exposing it on batch 0's critical
        # path
        warm_i = dram.tile([N_CORES, 16], BF16, tag="warm_i", name="warm_i")
        warm_o = dram.tile([N_CORES, 16], BF16, tag="warm_o", name="warm_o")
        # four quarter-exchanges (one per batch)
        in_b = [dram.tile([N_CORES * 128, 256], BF16, tag=f"in_b{i}",
                          name=f"in_b{i}") for i in range(B)]
        out_b = [dram.tile([N_CORES * 128, 256], BF16, tag=f"out_b{i}",
                           name=f"out_b{i}") for i in range(B)]
        # DRAM bounce for the reciprocal-denominator partition broadcast
        rcpd = dram.tile([1, B * NT * 2048], F32, tag="rcpd", name="rcpd")

        # --- constants ---
        triu = consts.tile([128, 128], BF16)
        make_upper_triangular(nc, triu, 1.0, diag=True)
        bq_sb = consts.tile([HC, 1], F32)
        nc.sync.dma_start(out=bq_sb, in_=bq[:, :])
        bk_sb = consts.tile([HC, 1], F32)
        nc.sync.dma_start(out=bk_sb, in_=bk[:, :])
        bv_sb = consts.tile([HC, 1], F32)
        nc.sync.dma_start(out=bv_sb, in_=bv[:, :])
        bo_sb = consts.tile([128, D], F32)
        nc.sync.dma_start(out=bo_sb, in_=_bcast(bo, 128, D))

        # --- small weights first (proj(0) needs them immediately) ---
        wq_sb = wpool.tile([128, DC, HC], BF16, tag="wq_sb")
        nc.sync.dma_start(out=wq_sb, in_=wq.rearrange("(c p) m -> p c m", p=128))
        wk_sb = wpool.tile([128, DC, HC], BF16, tag="wk_sb")
        nc.sync.dma_start(out=wk_sb, in_=wk.rearrange("(c p) m -> p c m", p=128))
        wv_sb = wpool.tile([128, DC, HC], BF16, tag="wv_sb")
        nc.sync.dma_start(out=wv_sb, in_=wv.rearrange("(c p) m -> p c m", p=128))

        def emit_xt_dma(b):
            xT = xtp.tile([128, DC, S], BF16, tag="xT", name=f"xT_{b}")
            for c in range(DC):
                nc.sync.dma_start(
                    out=xT[:, c, :],
                    in_=xt[c * 128:(c + 1) * 128, b * S:(b + 1) * S])
            return xT

        def proj_steps(b, xT):
            """Generator: K^T/Q^T/V^T projection + V DMA-transpose for batch
            b, yielded in PE-dense steps so attention(b-1) emission can
            interleave them."""
            kt = kqv.tile([HC, S], BF16, tag="kt", name=f"kt_{b}")
            qt_ = kqv.tile([HC, S], BF16, tag="qt", name=f"qt_{b}")
            vt = kqv.tile([HC, S], BF16, tag="vt", name=f"vt_{b}")
            vnat = kqv.tile([128, ST, HC], BF16, tag="vnat", name=f"vnat_{b}")
            for dst, w_sb, b_sb in ((kt, wk_sb, bk_sb), (qt_, wq_sb, bq_sb),
                                    (vt, wv_sb, bv_sb)):
                for s2 in range(S // 1024):
                    # one weight load per c serves both 512-chunks of the pair;
                    # yield every 2 c's (~4 MMs) so the interleaved attention
                    # strips never starve behind a long projection burst
                    pp = [ps_pp.tile([128, 512], F32, tag="pp",
                                     name=f"pp_{b}_{s2}_{u}")
                          for u in range(2)]
                    for c in range(DC):
                        for u in range(2):
                            s4 = 2 * s2 + u
                            nc.tensor.matmul(
                                pp[u], lhsT=w_sb[:, c, :],
                                rhs=xT[:, c, s4 * 512:(s4 + 1) * 512],
                                start=(c == 0), stop=(c == DC - 1))
                        if c % 2 == 1:
                            yield None
                    for u in range(2):
                        s4 = 2 * s2 + u
                        nc.vector.tensor_scalar_add(
                            dst[:, s4 * 512:(s4 + 1) * 512], pp[u], b_sb)
                    if dst is vt:
                        # V natural via DMA XBAR transpose, per 512-chunk so
                        # the sync queue never waits long on vt eviction
                        for u in range(2):
                            s4 = 2 * s2 + u
                            nc.sync.dma_start_transpose(
                                out=vnat[:, 4 * s4:4 * s4 + 4, :],
                                in_=vt[:, s4 * 512:(s4 + 1) * 512])
                    yield None
            # DVE copies into the per-head [V_h|1] layout
            vsb = kqv.tile([128, ST, H_LOC * 65], BF16, tag="vsb",
                           name=f"vsb_{b}")
            v4 = vsb.rearrange("p s (h o) -> p s h o", o=65)
            nc.vector.memset(v4[:, :, :, 64:65], 1.0)
            for h in range(H_LOC):
                nc.vector.tensor_copy(v4[:, :, h, 0:64],
                                      vnat[:, :, h * 64:(h + 1) * 64])
            yield None
            kqv_tiles[b] = (kt, qt_, vsb)

        def oproj_steps(b):
            """Generator: output projection for batch b's q-rows (after its
            AllToAll). DMAs ride the GpSimd queue, which already serializes
            behind the collective, so they never block the sync queue."""
            aT = atp.tile([128, DC, 256], BF16, tag="aT", name=f"aT_{b}")
            for c in range(DC):
                nc.sync.dma_start(out=aT[:, c, :],
                                  in_=out_b[b][c * 128:(c + 1) * 128, :])
            yield None
            for qt2 in range(2):
                pp = [ps_pp.tile([128, 512], F32, tag="pp",
                                 name=f"ppo_{b}_{qt2}_{nh}")
                      for nh in range(2)]
                for c in range(DC):
                    for nh in range(2):
                        nc.tensor.matmul(
                            pp[nh], lhsT=aT[:, c, qt2 * 128:qt2 * 128 + 128],
                            rhs=wo_sb[:, c, nh * 512:(nh + 1) * 512],
                            start=(c == 0), stop=(c == DC - 1))
                    if c % 2 == 1:
                        yield None
                for nh in range(2):
                    osb = opool.tile([128, 512], F32, tag="osb")
                    nc.vector.tensor_add(osb, pp[nh],
                                         bo_sb[:, nh * 512:(nh + 1) * 512])
                    nc.sync.dma_start(
                        out=out[b * 256 + qt2 * 128:b * 256 + qt2 * 128 + 128,
                                nh * 512:(nh + 1) * 512],
                        in_=osb)
                yield None

        def _paced(gen, credit_per_yield):
            """Wrap a generator so each next() only advances it
            `credit_per_yield` steps on average — spreads interleaved work
            evenly across the attention strips instead of front-loading."""
            credit = 0.0
            while True:
                credit += credit_per_yield
                while credit >= 1.0:
                    credit -= 1.0
                    try:
                        next(gen)
                    except StopIteration:
                        return
                yield None

        def emit_attention(b, interleave):
            kt, qt_, vsb = kqv_tiles[b]
            for t in range(NT):
                q0 = 512 * t
                nj = 4 * t + 4
                po = [ps_po.tile([65, 512], F32, tag="po",
                                 name=f"po_{b}_{t}_{h}") for h in range(2)]

                def emit_pv(pend, po=po, nj=nj):
                    es_p, jp, relp = pend
                    for h in range(H_LOC):
                        nc.tensor.matmul(
                            po[h][:, relp:512],
                            lhsT=vsb[:, jp, h * 65:(h + 1) * 65],
                            rhs=es_p[:, h * 512 + relp:h * 512 + 512],
                            start=(jp == 0), stop=(jp == nj - 1),
                            skip_group_check=True)

                prev = None
                for j in range(nj):
                    rel = max(0, 128 * j - q0)
                    ps = ps_s.tile([128, 1024], F32, tag="ps")
                    # both heads' scores concurrently via PE row-group tiles
                    nc.tensor.matmul(ps[:, rel:512],
                                     lhsT=kt[0:64, j * 128:(j + 1) * 128],
                                     rhs=qt_[0:64, q0 + rel:q0 + 512],
                                     start=True, stop=True)
                    nc.tensor.matmul(ps[:, 512 + rel:1024],
                                     lhsT=kt[64:128, j * 128:(j + 1) * 128],
                                     rhs=qt_[64:128, q0 + rel:q0 + 512],
                                     start=True, stop=True,
                                     skip_group_check=True)
                    es = epool.tile([128, 1024], BF16, tag="et")
                    # one wide exp covers both heads ([512:512+rel) is unused
                    # garbage on diagonal strips, never consumed by PV)
                    nc.scalar.activation(es[:, rel:1024], ps[:, rel:1024],
                                         mybir.ActivationFunctionType.Exp,
                                         scale=1.0 / np.sqrt(DK))
                    if 128 * j >= q0:  # diagonal strip: causal 0/1 mask
                        nc.vector.tensor_mul(es[:, rel:rel + 128],
                                             es[:, rel:rel + 128], triu)
                        nc.vector.tensor_mul(es[:, 512 + rel:512 + rel + 128],
                                             es[:, 512 + rel:512 + rel + 128],
                                             triu)
                    if prev:
                        emit_pv(prev)
                        next(interleave, None)
                    prev = (es, j, rel)
                    next(interleave, None)
                emit_pv(prev)
                next(interleave, None)

                # normalize (no transposes: po is already [head-dim, q]).
                # DVE copy to SBUF releases the po PSUM banks immediately.
                # The denominator row lives on ONE partition, where a DVE
                # reciprocal would be serial (8 cyc/elem = 3.4us); instead
                # DMA-spread it across 64 lanes, reciprocal there (0.1us),
                # gather back and broadcast-read. The whole chain rides the
                # GpSimd DMA queue so it never blocks sync-queue traffic.
                poc = rpool.tile([65, 1024], F32, tag="poc")
                nc.vector.tensor_copy(poc[:, 0:512], po[0])
                nc.vector.tensor_copy(poc[:, 512:1024], po[1])
                slot = (4 * b + t) * 2048
                nc.gpsimd.dma_start(out=rcpd[0:1, slot:slot + 1024],
                                    in_=poc[64:65, :])
                dsc = spool.tile([64, 16], F32, tag="dsc")
                nc.gpsimd.dma_start(
                    out=dsc,
                    in_=rcpd[0:1, slot:slot + 1024].rearrange(
                        "a (b c) -> (a b) c", c=16))
                rcv = spool.tile([64, 16], F32, tag="rcv")
                nc.vector.reciprocal(rcv, dsc)
                nc.gpsimd.dma_start(
                    out=rcpd[0:1, slot + 1024:slot + 2048].rearrange(
                        "a (b c) -> (a b) c", c=16),
                    in_=rcv)
                rb = bpool.tile([64, 1024], F32, tag="rb")
                nc.gpsimd.dma_start(
                    out=rb,
                    in_=rcpd[0:1, slot + 1024:slot + 2048].to_broadcast(
                        [64, 1024]))
                for h in range(H_LOC):
                    st = spool.tile([64, 512], BF16, tag="st")
                    nc.vector.tensor_mul(st, poc[0:64, h * 512:(h + 1) * 512],
                                         rb[0:64, h * 512:(h + 1) * 512])
                    for u in range(2):
                        s = 2 * t + u
                        nc.gpsimd.dma_start(
                            out=in_b[b][s * 128 + h * 64:s * 128 + h * 64 + 64,
                                        0:256],
                            in_=st[:, u * 256:(u + 1) * 256])
                next(interleave, None)

            nc.gpsimd.collective_compute(
                "AllToAll", mybir.AluOpType.bypass,
                replica_groups=[list(range(N_CORES))],
                ins=[in_b[b].opt()], outs=[out_b[b].opt()])

        # warm up the collective path while the pipeline head loads/projects
        warm_sb = consts.tile([N_CORES, 16], BF16)
        nc.sync.dma_start(out=warm_sb, in_=xt[0:N_CORES, 0:16])
        nc.gpsimd.dma_start(out=warm_i[:, :], in_=warm_sb)
        nc.gpsimd.collective_compute(
            "AllToAll", mybir.AluOpType.bypass,
            replica_groups=[list(range(N_CORES))],
            ins=[warm_i.opt()], outs=[warm_o.opt()])

        kqv_tiles = {}
        xT0 = emit_xt_dma(0)
        # wo is only needed by oproj(0), well after proj(0): load it after
        # xT(0) so it doesn't delay the pipeline head on the sync DMA queue
        wo_sb = wpool.tile([128, DC, D], BF16, tag="wo_sb")
        nc.sync.dma_start(out=wo_sb, in_=wo.rearrange("(c p) m -> p c m", p=128))

        for _ in proj_steps(0, xT0):
            pass
        pending = []
        for b in range(B):
            gens = []
            if b + 1 < B:
                xTn = emit_xt_dma(b + 1)
                gens.append(proj_steps(b + 1, xTn))
            gens.extend(pending)
            pending = []
            # ~42 interleave steps spread across ~84 attention yield points
            inter = _paced(chain(*gens), 0.5) if gens else iter(())
            emit_attention(b, inter)
            for _ in inter:
                pass
            pending = [oproj_steps(b)]
        for g in pending:
            for _ in g:
                pass

    nc.compile()
    return nc


_NC_CACHE = None


def _get_program():
    global _NC_CACHE
    if _NC_CACHE is None:
        _NC_CACHE = build_program()
    return _NC_CACHE


def _make_in_maps(x, w_qkv, b_qkv, w_o, b_o):
    x = np.asarray(x, dtype=np.float32).reshape(B * S, D)
    xt = np.ascontiguousarray(x.T).astype(BF16_NP)
    w_qkv = np.asarray(w_qkv, dtype=np.float32)
    b_qkv = np.asarray(b_qkv, dtype=np.float32)
    wo_bf = np.ascontiguousarray(np.asarray(w_o, dtype=np.float32)).astype(BF16_NP)
    b_o = np.asarray(b_o, dtype=np.float32).reshape(1, D)
    in_maps = []
    for c in range(N_CORES):
        lo = c * HC
        hi = lo + HC
        in_maps.append({
            "xt": xt,
            "wq": np.ascontiguousarray(w_qkv[:, lo:hi]).astype(BF16_NP),
            "wk": np.ascontiguousarray(w_qkv[:, D + lo:D + hi]).astype(BF16_NP),
            "wv": np.ascontiguousarray(w_qkv[:, 2 * D + lo:2 * D + hi]).astype(BF16_NP),
            "bq": np.ascontiguousarray(b_qkv[lo:hi].reshape(HC, 1)),
            "bk": np.ascontiguousarray(b_qkv[D + lo:D + hi].reshape(HC, 1)),
            "bv": np.ascontiguousarray(b_qkv[2 * D + lo:2 * D + hi].reshape(HC, 1)),
            "wo": wo_bf,
            "bo": b_o,
        })
    return in_maps


def _assemble(results):
    out = np.empty((B, S, D), dtype=np.float32)
    for c in range(N_CORES):
        q0 = c * 256
        for k in range(B):
            out[k, q0:q0 + 256, :] = results[c]["out"][k * 256:(k + 1) * 256]
    return out


def run(x, mask, w_qkv, b_qkv, w_o, b_o, trace=False, **trace_kwargs):
    """Run on hardware; returns (output, BassKernelResults)."""
    nc = _get_program()
    in_maps = _make_in_maps(x, w_qkv, b_qkv, w_o, b_o)
    res = run_bass_kernel_spmd(nc, in_maps, list(range(N_CORES)),
                               trace=trace, **trace_kwargs)
    return _assemble(res.results), res


def kernel(x, mask, w_qkv, b_qkv, w_o, b_o):
    out, _ = run(x, mask, w_qkv, b_qkv, w_o, b_o)
    return out


# revision 24
# speedup vs baseline: 1.0221x; 1.0221x over previous
"""MultiHeadAttention (B=4, S=2048, D=1024, H=16, causal) on 8 TRN2 NeuronCores.

Sharding: tensor-parallel over heads across all 8 cores (2 heads/core, all 4
batches processed locally; identical SPMD control flow on every core). After
attention, an 8-core AllToAll per batch redistributes attention outputs so
each core runs the output projection for 1/8 of the (batch, seq) rows.

Per-core pipeline (all matmuls bf16 with f32 PSUM accumulation):
  - x arrives host-transposed as x^T [D, B*S] bf16; K^T/Q^T/V^T via
    w-stationary matmuls (each weight load serves a pair of 512-wide chunks),
    bias added on the DVE eviction. V^T is transposed to natural V by the DMA
    XBAR engine (no PE involvement) with a ones column per head so the PV
    matmul also produces the softmax denominator.
  - Scores are computed transposed ([k, q] = K @ Q^T) in 512-wide q-chunks.
    The two heads' score matmuls are issued as row-group tiles (K=64 each,
    rows 0-63 / 64-127) so they execute CONCURRENTLY in the PE array and land
    in the two PSUM banks of one [128, 1024] tile; a single wide exp on
    ScalarE covers both heads (1/sqrt(dk) scale folded in; no max subtraction
    needed: |scores| <~ 2.6). Causal mask = 0/1 triangular multiply on
    diagonal tiles only; fully-masked tiles are never computed.
  - PV: [V_h|1] stationary, exp chunks stream, accumulating po_h = [out^T;
    denom] [65, 512] in PSUM, one k-strip behind scores/exp so the in-order
    PE queue never head-of-line blocks on ScalarE.
  - po is already in the [head-dim, q] layout the AllToAll needs, so there
    are no output transposes at all: the denominator row is reciprocal'd,
    bounced through DRAM to broadcast it across 64 partitions, and a single
    DVE multiply writes the normalized bf16 attention output.
  - Output projection for batch b runs right after its AllToAll, interleaved
    into the next batch's attention (its DMAs ride the GpSimd queue, which
    already serializes behind the collective).
"""

import sys

if "/opt/trn_rl_repo" not in sys.path:
    sys.path.insert(0, "/opt/trn_rl_repo")

from contextlib import ExitStack
from itertools import chain

import ml_dtypes
import numpy as np

import concourse.bacc as bacc
import concourse.bass as bass
import concourse.mybir as mybir
import concourse.tile as tile
from concourse.bass_utils import run_bass_kernel_spmd
from concourse.masks import make_upper_triangular

N_CORES = 8
B = 4
S = 2048
D = 1024
H_TOT = 16
DK = 64
H_LOC = H_TOT // N_CORES  # 2 heads per core
HC = H_LOC * DK  # 128 head-cols per core
ST = S // 128  # 16 k-strips per batch
DC = D // 128  # 8 d_model chunks
NT = S // 512  # 4 q-chunks per batch
BQ = (B * S) // N_CORES  # 1024 (batch,seq) rows per core after AllToAll

F32 = mybir.dt.float32
BF16 = mybir.dt.bfloat16
BF16_NP = ml_dtypes.bfloat16


def _bcast(handle, rows, cols):
    """AP reading a [1, cols] DRAM tensor broadcast over `rows` partitions."""
    return bass.AP(tensor=handle, offset=0, ap=[[0, rows], [1, cols]])


def build_program():
    nc = bacc.Bacc("TRN2", target_bir_lowering=False, debug=False,
                   num_devices=N_CORES)

    xt = nc.declare_dram_parameter("xt", [D, B * S], BF16, isOutput=False)
    wq = nc.declare_dram_parameter("wq", [D, HC], BF16, isOutput=False)
    wk = nc.declare_dram_parameter("wk", [D, HC], BF16, isOutput=False)
    wv = nc.declare_dram_parameter("wv", [D, HC], BF16, isOutput=False)
    bq = nc.declare_dram_parameter("bq", [HC, 1], F32, isOutput=False)
    bk = nc.declare_dram_parameter("bk", [HC, 1], F32, isOutput=False)
    bv = nc.declare_dram_parameter("bv", [HC, 1], F32, isOutput=False)
    wo = nc.declare_dram_parameter("wo", [D, D], BF16, isOutput=False)
    bo = nc.declare_dram_parameter("bo", [1, D], F32, isOutput=False)
    out = nc.declare_dram_parameter("out", [BQ, D], F32, isOutput=True)

    with ExitStack() as ctx:
        tc = ctx.enter_context(tile.TileContext(nc))

        consts = ctx.enter_context(tc.tile_pool(name="consts", bufs=1))
        wpool = ctx.enter_context(tc.tile_pool(name="wpool", bufs=1))
        xtp = ctx.enter_context(tc.tile_pool(name="xtp", bufs=2))
        kqv = ctx.enter_context(tc.tile_pool(name="kqv", bufs=2))
        epool = ctx.enter_context(tc.tile_pool(name="epool", bufs=4))
        # normalize-chain pools are deep enough (4 chunks = one full batch)
        # to ride out the first-collective setup (~56us) blocking the GpSimd
        # DMA queue without backpressuring the PV/exp pipeline
        rpool = ctx.enter_context(tc.tile_pool(name="rpool", bufs=5))
        bpool = ctx.enter_context(tc.tile_pool(name="bpool", bufs=5))
        spool = ctx.enter_context(tc.tile_pool(name="spool", bufs=10))
        opool = ctx.enter_context(tc.tile_pool(name="opool", bufs=2))
        atp = ctx.enter_context(tc.tile_pool(name="atp", bufs=2))
        ps_s = ctx.enter_context(tc.tile_pool(name="ps_s", bufs=2, space="PSUM"))
        ps_po = ctx.enter_context(tc.tile_pool(name="ps_po", bufs=2, space="PSUM"))
        ps_pp = ctx.enter_context(tc.tile_pool(name="ps_pp", bufs=2, space="PSUM"))
        dram = ctx.enter_context(tc.tile_pool(name="dram", bufs=1, space="DRAM"))

        # tiny dummy exchange issued at kernel start: absorbs the one-time
        # first-collective setup (~40us) concurrently with the initial
        # DMA/projection phase instead of exposing it on batch 0's critical
        # path
        warm_i = dram.tile([N_CORES, 16], BF16, tag="warm_i", name="warm_i")
        warm_o = dram.tile([N_CORES, 16], BF16, tag="warm_o", name="warm_o")
        # four quarter-exchanges (one per batch)
        in_b = [dram.tile([N_CORES * 128, 256], BF16, tag=f"in_b{i}",
                          name=f"in_b{i}") for i in range(B)]
        out_b = [dram.tile([N_CORES * 128, 256], BF16, tag=f"out_b{i}",
                           name=f"out_b{i}") for i in range(B)]
        # DRAM bounce for the reciprocal-denominator partition broadcast
        rcpd = dram.tile([1, B * NT * 2048], F32, tag="rcpd", name="rcpd")

        # --- constants ---
        triu = consts.tile([128, 128], BF16)
        make_upper_triangular(nc, triu, 1.0, diag=True)
        bq_sb = consts.tile([HC, 1], F32)
        nc.sync.dma_start(out=bq_sb, in_=bq[:, :])
        bk_sb = consts.tile([HC, 1], F32)
        nc.sync.dma_start(out=bk_sb, in_=bk[:, :])
        bv_sb = consts.tile([HC, 1], F32)
        nc.sync.dma_start(out=bv_sb, in_=bv[:, :])
        bo_sb = consts.tile([128, D], F32)
        nc.sync.dma_start(out=bo_sb, in_=_bcast(bo, 128, D))

        # --- small weights first (proj(0) needs them immediately) ---
        wq_sb = wpool.tile([128, DC, HC], BF16, tag="wq_sb")
        nc.sync.dma_start(out=wq_sb, in_=wq.rearrange("(c p) m -> p c m", p=128))
        wk_sb = wpool.tile([128, DC, HC], BF16, tag="wk_sb")
        nc.sync.dma_start(out=wk_sb, in_=wk.rearrange("(c p) m -> p c m", p=128))
        wv_sb = wpool.tile([128, DC, HC], BF16, tag="wv_sb")
        nc.sync.dma_start(out=wv_sb, in_=wv.rearrange("(c p) m -> p c m", p=128))

        def emit_xt_dma(b):
            xT = xtp.tile([128, DC, S], BF16, tag="xT", name=f"xT_{b}")
            for c in range(DC):
                nc.sync.dma_start(
                    out=xT[:, c, :],
                    in_=xt[c * 128:(c + 1) * 128, b * S:(b + 1) * S])
            return xT

        def proj_steps(b, xT):
            """Generator: K^T/Q^T/V^T projection + V DMA-transpose for batch
            b, yielded in PE-dense steps so attention(b-1) emission can
            interleave them."""
            kt = kqv.tile([HC, S], BF16, tag="kt", name=f"kt_{b}")
            qt_ = kqv.tile([HC, S], BF16, tag="qt", name=f"qt_{b}")
            vt = kqv.tile([HC, S], BF16, tag="vt", name=f"vt_{b}")
            vnat = kqv.tile([128, ST, HC], BF16, tag="vnat", name=f"vnat_{b}")
            for dst, w_sb, b_sb in ((kt, wk_sb, bk_sb), (qt_, wq_sb, bq_sb),
                                    (vt, wv_sb, bv_sb)):
                for s2 in range(S // 1024):
                    # one weight load per c serves both 512-chunks of the pair;
                    # yield every 2 c's (~4 MMs) so the interleaved attention
                    # strips never starve behind a long projection burst
                    pp = [ps_pp.tile([128, 512], F32, tag="pp",
                                     name=f"pp_{b}_{s2}_{u}")
                          for u in range(2)]
                    for c in range(DC):
                        for u in range(2):
                            s4 = 2 * s2 + u
                            nc.tensor.matmul(
                                pp[u], lhsT=w_sb[:, c, :],
                                rhs=xT[:, c, s4 * 512:(s4 + 1) * 512],
                                start=(c == 0), stop=(c == DC - 1))
                        if c % 2 == 1:
                            yield None
                    for u in range(2):
                        s4 = 2 * s2 + u
                        nc.vector.tensor_scalar_add(
                            dst[:, s4 * 512:(s4 + 1) * 512], pp[u], b_sb)
                    if dst is vt:
                        # V natural via DMA XBAR transpose, per 512-chunk so
                        # the sync queue never waits long on vt eviction
                        for u in range(2):
                            s4 = 2 * s2 + u
                            nc.sync.dma_start_transpose(
                                out=vnat[:, 4 * s4:4 * s4 + 4, :],
                                in_=vt[:, s4 * 512:(s4 + 1) * 512])
                    yield None
            # DVE copies into the per-head [V_h|1] layout
            vsb = kqv.tile([128, ST, H_LOC * 65], BF16, tag="vsb",
                           name=f"vsb_{b}")
            v4 = vsb.rearrange("p s (h o) -> p s h o", o=65)
            nc.vector.memset(v4[:, :, :, 64:65], 1.0)
            for h in range(H_LOC):
                nc.vector.tensor_copy(v4[:, :, h, 0:64],
                                      vnat[:, :, h * 64:(h + 1) * 64])
            yield None
            kqv_tiles[b] = (kt, qt_, vsb)

        def oproj_steps(b):
            """Generator: output projection for batch b's q-rows (after its
            AllToAll). DMAs ride the GpSimd queue, which already serializes
            behind the collective, so they never block the sync queue."""
            aT = atp.tile([128, DC, 256], BF16, tag="aT", name=f"aT_{b}")
            for c in range(DC):
                nc.sync.dma_start(out=aT[:, c, :],
                                  in_=out_b[b][c * 128:(c + 1) * 128, :])
            yield None
            for qt2 in range(2):
                pp = [ps_pp.tile([128, 512], F32, tag="pp",
                                 name=f"ppo_{b}_{qt2}_{nh}")
                      for nh in range(2)]
                for c in range(DC):
                    for nh in range(2):
                        nc.tensor.matmul(
                            pp[nh], lhsT=aT[:, c, qt2 * 128:qt2 * 128 + 128],
                            rhs=wo_sb[:, c, nh * 512:(nh + 1) * 512],
                            start=(c == 0), stop=(c == DC - 1))
                    if c % 2 == 1:
                        yield None
                for nh in range(2):
                    osb = opool.tile([128, 512], F32, tag="osb")
                    nc.vector.tensor_add(osb, pp[nh],
                                         bo_sb[:, nh * 512:(nh + 1) * 512])
                    nc.sync.dma_start(
                        out=out[b * 256 + qt2 * 128:b * 256 + qt2 * 128 + 128,
                                nh * 512:(nh + 1) * 512],
                        in_=osb)
                yield None

        def _paced(gen, credit_per_yield):
            """Wrap a generator so each next() only advances it
            `credit_per_yield` steps on average — spreads interleaved work
            evenly across the attention strips instead of front-loading."""
            credit = 0.0
            while True:
                credit += credit_per_yield
                while credit >= 1.0:
                    credit -= 1.0
                    try:
                        next(gen)
                    except StopIteration:
                        return
                yield None

        def emit_attention(b, interleave):
            kt, qt_, vsb = kqv_tiles[b]
            for t in range(NT):
                q0 = 512 * t
                nj = 4 * t + 4
                po = [ps_po.tile([65, 512], F32, tag="po",
                                 name=f"po_{b}_{t}_{h}") for h in range(2)]

                def emit_pv(pend, po=po, nj=nj):
                    es_p, jp, relp = pend
                    for h in range(H_LOC):
                        nc.tensor.matmul(
                            po[h][:, relp:512],
                            lhsT=vsb[:, jp, h * 65:(h + 1) * 65],
                            rhs=es_p[:, h * 512 + relp:h * 512 + 512],
                            start=(jp == 0), stop=(jp == nj - 1),
                            skip_group_check=True)

                prev = None
                for j in range(nj):
                    rel = max(0, 128 * j - q0)
                    ps = ps_s.tile([128, 1024], F32, tag="ps")
                    # both heads' scores concurrently via PE row-group tiles
                    nc.tensor.matmul(ps[:, rel:512],
                                     lhsT=kt[0:64, j * 128:(j + 1) * 128],
                                     rhs=qt_[0:64, q0 + rel:q0 + 512],
                                     start=True, stop=True)
                    nc.tensor.matmul(ps[:, 512 + rel:1024],
                                     lhsT=kt[64:128, j * 128:(j + 1) * 128],
                                     rhs=qt_[64:128, q0 + rel:q0 + 512],
                                     start=True, stop=True,
                                     skip_group_check=True)
                    es = epool.tile([128, 1024], BF16, tag="et")
                    # one wide exp covers both heads ([512:512+rel) is unused
                    # garbage on diagonal strips, never consumed by PV)
                    nc.scalar.activation(es[:, rel:1024], ps[:, rel:1024],
                                         mybir.ActivationFunctionType.Exp,
                                         scale=1.0 / np.sqrt(DK))
                    if 128 * j >= q0:  # diagonal strip: causal 0/1 mask
                        nc.vector.tensor_mul(es[:, rel:rel + 128],
                                             es[:, rel:rel + 128], triu)
                        nc.vector.tensor_mul(es[:, 512 + rel:512 + rel + 128],
                                             es[:, 512 + rel:512 + rel + 128],
                                             triu)
                    if prev:
                        emit_pv(prev)
                        next(interleave, None)
                    prev = (es, j, rel)
                    next(interleave, None)
                emit_pv(prev)
                next(interleave, None)

                # normalize (no transposes: po is already [head-dim, q]).
                # DVE copy to SBUF releases the po PSUM banks immediately.
                # The denominator row lives on ONE partition, where a DVE
                # reciprocal would be serial (8 cyc/elem = 3.4us); instead
                # DMA-spread it across 64 lanes, reciprocal there (0.1us),
                # gather back and broadcast-read. The whole chain rides the
                # GpSimd DMA queue so it never blocks sync-queue traffic.
                poc = rpool.tile([65, 1024], F32, tag="poc")
                nc.vector.tensor_copy(poc[:, 0:512], po[0])
                nc.vector.tensor_copy(poc[:, 512:1024], po[1])
                slot = (4 * b + t) * 2048
                nc.gpsimd.dma_start(out=rcpd[0:1, slot:slot + 1024],
                                    in_=poc[64:65, :])
                dsc = spool.tile([64, 16], F32, tag="dsc")
                nc.gpsimd.dma_start(
                    out=dsc,
                    in_=rcpd[0:1, slot:slot + 1024].rearrange(
                        "a (b c) -> (a b) c", c=16))
                rcv = spool.tile([64, 16], F32, tag="rcv")
                nc.vector.reciprocal(rcv, dsc)
                nc.gpsimd.dma_start(
                    out=rcpd[0:1, slot + 1024:slot + 2048].rearrange(
                        "a (b c) -> (a b) c", c=16),
                    in_=rcv)
                rb = bpool.tile([64, 1024], F32, tag="rb")
                nc.gpsimd.dma_start(
                    out=rb,
                    in_=rcpd[0:1, slot + 1024:slot + 2048].to_broadcast(
                        [64, 1024]))
                for h in range(H_LOC):
                    # normalize-mul runs on GpSimd: it serializes after its
                    # rb load on the same queue, so a late rb never blocks
                    # the DVE queue (whose mask-muls gate the PE pipeline)
                    st = spool.tile([64, 512], BF16, tag="st")
                    nc.gpsimd.tensor_mul(st, poc[0:64, h * 512:(h + 1) * 512],
                                         rb[0:64, h * 512:(h + 1) * 512])
                    for u in range(2):
                        s = 2 * t + u
                        nc.gpsimd.dma_start(
                            out=in_b[b][s * 128 + h * 64:s * 128 + h * 64 + 64,
                                        0:256],
                            in_=st[:, u * 256:(u + 1) * 256])
                next(interleave, None)

            nc.gpsimd.collective_compute(
                "AllToAll", mybir.AluOpType.bypass,
                replica_groups=[list(range(N_CORES))],
                ins=[in_b[b].opt()], outs=[out_b[b].opt()])

        # warm up the collective path while the pipeline head loads/projects
        warm_sb = consts.tile([N_CORES, 16], BF16)
        nc.sync.dma_start(out=warm_sb, in_=xt[0:N_CORES, 0:16])
        nc.gpsimd.dma_start(out=warm_i[:, :], in_=warm_sb)
        nc.gpsimd.collective_compute(
            "AllToAll", mybir.AluOpType.bypass,
            replica_groups=[list(range(N_CORES))],
            ins=[warm_i.opt()], outs=[warm_o.opt()])

        kqv_tiles = {}
        xT0 = emit_xt_dma(0)
        # wo is only needed by oproj(0), well after proj(0): load it after
        # xT(0) so it doesn't delay the pipeline head on the sync DMA queue
        wo_sb = wpool.tile([128, DC, D], BF16, tag="wo_sb")
        nc.sync.dma_start(out=wo_sb, in_=wo.rearrange("(c p) m -> p c m", p=128))

        for _ in proj_steps(0, xT0):
            pass
        pending = []
        for b in range(B):
            gens = []
            if b + 1 < B:
                xTn = emit_xt_dma(b + 1)
                gens.append(proj_steps(b + 1, xTn))
            gens.extend(pending)
            pending = []
            # ~42 interleave steps across ~84 attention yield points; run
            # slightly ahead so proj(b+1) is done before attention(b+1)
            inter = _paced(chain(*gens), 1.0) if gens else iter(())
            emit_attention(b, inter)
            for _ in inter:
                pass
            pending = [oproj_steps(b)]
        for g in pending:
            for _ in g:
                pass

    nc.compile()
    return nc


_NC_CACHE = None


def _get_program():
    global _NC_CACHE
    if _NC_CACHE is None:
        _NC_CACHE = build_program()
    return _NC_CACHE


def _make_in_maps(x, w_qkv, b_qkv, w_o, b_o):
    x = np.asarray(x, dtype=np.float32).reshape(B * S, D)
    xt = np.ascontiguousarray(x.T).astype(BF16_NP)
    w_qkv = np.asarray(w_qkv, dtype=np.float32)
    b_qkv = np.asarray(b_qkv, dtype=np.float32)
    wo_bf = np.ascontiguousarray(np.asarray(w_o, dtype=np.float32)).astype(BF16_NP)
    b_o = np.asarray(b_o, dtype=np.float32).reshape(1, D)
    in_maps = []
    for c in range(N_CORES):
        lo = c * HC
        hi = lo + HC
        in_maps.append({
            "xt": xt,
            "wq": np.ascontiguousarray(w_qkv[:, lo:hi]).astype(BF16_NP),
            "wk": np.ascontiguousarray(w_qkv[:, D + lo:D + hi]).astype(BF16_NP),
            "wv": np.ascontiguousarray(w_qkv[:, 2 * D + lo:2 * D + hi]).astype(BF16_NP),
            "bq": np.ascontiguousarray(b_qkv[lo:hi].reshape(HC, 1)),
            "bk": np.ascontiguousarray(b_qkv[D + lo:D + hi].reshape(HC, 1)),
            "bv": np.ascontiguousarray(b_qkv[2 * D + lo:2 * D + hi].reshape(HC, 1)),
            "wo": wo_bf,
            "bo": b_o,
        })
    return in_maps


def _assemble(results):
    out = np.empty((B, S, D), dtype=np.float32)
    for c in range(N_CORES):
        q0 = c * 256
        for k in range(B):
            out[k, q0:q0 + 256, :] = results[c]["out"][k * 256:(k + 1) * 256]
    return out


def run(x, mask, w_qkv, b_qkv, w_o, b_o, trace=False, **trace_kwargs):
    """Run on hardware; returns (output, BassKernelResults)."""
    nc = _get_program()
    in_maps = _make_in_maps(x, w_qkv, b_qkv, w_o, b_o)
    res = run_bass_kernel_spmd(nc, in_maps, list(range(N_CORES)),
                               trace=trace, **trace_kwargs)
    return _assemble(res.results), res


def kernel(x, mask, w_qkv, b_qkv, w_o, b_o):
    out, _ = run(x, mask, w_qkv, b_qkv, w_o, b_o)
    return out


# revision 31
# speedup vs baseline: 1.1003x; 1.0765x over previous
"""MultiHeadAttention (B=4, S=2048, D=1024, H=16, causal) on 8 TRN2 NeuronCores.

Sharding: tensor-parallel over heads across all 8 cores (2 heads/core, all 4
batches processed locally; identical SPMD control flow on every core). After
attention, an 8-core AllToAll per batch redistributes attention outputs so
each core runs the output projection for 1/8 of the (batch, seq) rows.

Per-core pipeline (all matmuls bf16 with f32 PSUM accumulation):
  - x arrives host-transposed as x^T [D, B*S] bf16; K^T/Q^T/V^T via
    w-stationary matmuls (each weight load serves a pair of 512-wide chunks),
    bias added on the DVE eviction. V^T is transposed to natural V by the DMA
    XBAR engine (no PE involvement) with a ones column per head so the PV
    matmul also produces the softmax denominator.
  - Scores are computed transposed ([k, q] = K @ Q^T) in 512-wide q-chunks.
    The two heads' score matmuls are issued as row-group tiles (K=64 each,
    rows 0-63 / 64-127) so they execute CONCURRENTLY in the PE array and land
    in the two PSUM banks of one [128, 1024] tile; a single wide exp on
    ScalarE covers both heads (1/sqrt(dk) scale folded in; no max subtraction
    needed: |scores| <~ 2.6). Causal mask = 0/1 triangular multiply on
    diagonal tiles only; fully-masked tiles are never computed.
  - PV: [V_h|1] stationary, exp chunks stream, accumulating po_h = [out^T;
    denom] [65, 512] in PSUM, one k-strip behind scores/exp so the in-order
    PE queue never head-of-line blocks on ScalarE.
  - po is already in the [head-dim, q] layout the AllToAll needs, so there
    are no output transposes at all: the denominator row is reciprocal'd,
    bounced through DRAM to broadcast it across 64 partitions, and a single
    DVE multiply writes the normalized bf16 attention output.
  - Output projection for batch b runs right after its AllToAll, interleaved
    into the next batch's attention (its DMAs ride the GpSimd queue, which
    already serializes behind the collective).
"""

import sys

if "/opt/trn_rl_repo" not in sys.path:
    sys.path.insert(0, "/opt/trn_rl_repo")

from contextlib import ExitStack
from itertools import chain

import ml_dtypes
import numpy as np

import concourse.bacc as bacc
import concourse.bass as bass
import concourse.mybir as mybir
import concourse.tile as tile
from concourse.bass_utils import run_bass_kernel_spmd
from concourse.masks import make_identity, make_upper_triangular

N_CORES = 8
B = 4
S = 2048
D = 1024
H_TOT = 16
DK = 64
H_LOC = H_TOT // N_CORES  # 2 heads per core
HC = H_LOC * DK  # 128 head-cols per core
ST = S // 128  # 16 k-strips per batch
DC = D // 128  # 8 d_model chunks
NT = S // 512  # 4 q-chunks per batch
BQ = (B * S) // N_CORES  # 1024 (batch,seq) rows per core after AllToAll

F32 = mybir.dt.float32
BF16 = mybir.dt.bfloat16
BF16_NP = ml_dtypes.bfloat16


def _bcast(handle, rows, cols):
    """AP reading a [1, cols] DRAM tensor broadcast over `rows` partitions."""
    return bass.AP(tensor=handle, offset=0, ap=[[0, rows], [1, cols]])


def build_program():
    nc = bacc.Bacc("TRN2", target_bir_lowering=False, debug=False,
                   num_devices=N_CORES)

    xt = nc.declare_dram_parameter("xt", [D, B * S], BF16, isOutput=False)
    wq = nc.declare_dram_parameter("wq", [D, HC], BF16, isOutput=False)
    wk = nc.declare_dram_parameter("wk", [D, HC], BF16, isOutput=False)
    wv = nc.declare_dram_parameter("wv", [D, HC], BF16, isOutput=False)
    bq = nc.declare_dram_parameter("bq", [HC, 1], F32, isOutput=False)
    bk = nc.declare_dram_parameter("bk", [HC, 1], F32, isOutput=False)
    bv = nc.declare_dram_parameter("bv", [HC, 1], F32, isOutput=False)
    wo = nc.declare_dram_parameter("wo", [D, D], BF16, isOutput=False)
    bo = nc.declare_dram_parameter("bo", [1, D], F32, isOutput=False)
    out = nc.declare_dram_parameter("out", [BQ, D], F32, isOutput=True)

    with ExitStack() as ctx:
        tc = ctx.enter_context(tile.TileContext(nc))

        consts = ctx.enter_context(tc.tile_pool(name="consts", bufs=1))
        wpool = ctx.enter_context(tc.tile_pool(name="wpool", bufs=1))
        xtp = ctx.enter_context(tc.tile_pool(name="xtp", bufs=2))
        kqv = ctx.enter_context(tc.tile_pool(name="kqv", bufs=2))
        epool = ctx.enter_context(tc.tile_pool(name="epool", bufs=4))
        # normalize-chain pools are deep enough (4 chunks = one full batch)
        # to ride out the first-collective setup (~56us) blocking the GpSimd
        # DMA queue without backpressuring the PV/exp pipeline
        rpool = ctx.enter_context(tc.tile_pool(name="rpool", bufs=5))
        bpool = ctx.enter_context(tc.tile_pool(name="bpool", bufs=5))
        spool = ctx.enter_context(tc.tile_pool(name="spool", bufs=10))
        opool = ctx.enter_context(tc.tile_pool(name="opool", bufs=2))
        atp = ctx.enter_context(tc.tile_pool(name="atp", bufs=2))
        ps_s = ctx.enter_context(tc.tile_pool(name="ps_s", bufs=2, space="PSUM"))
        ps_po = ctx.enter_context(tc.tile_pool(name="ps_po", bufs=2, space="PSUM"))
        ps_pp = ctx.enter_context(tc.tile_pool(name="ps_pp", bufs=2, space="PSUM"))
        dram = ctx.enter_context(tc.tile_pool(name="dram", bufs=1, space="DRAM"))

        # tiny dummy exchange issued at kernel start: absorbs the one-time
        # first-collective setup (~40us) concurrently with the initial
        # DMA/projection phase instead of exposing it on batch 0's critical
        # path
        warm_i = dram.tile([N_CORES, 16], BF16, tag="warm_i", name="warm_i")
        warm_o = dram.tile([N_CORES, 16], BF16, tag="warm_o", name="warm_o")
        # four quarter-exchanges (one per batch)
        in_b = [dram.tile([N_CORES * 128, 256], BF16, tag=f"in_b{i}",
                          name=f"in_b{i}") for i in range(B)]
        out_b = [dram.tile([N_CORES * 128, 256], BF16, tag=f"out_b{i}",
                           name=f"out_b{i}") for i in range(B)]
        # DRAM bounce for the reciprocal-denominator partition broadcast
        rcpd = dram.tile([1, B * NT * 2048], F32, tag="rcpd", name="rcpd")

        # --- constants ---
        triu = consts.tile([128, 128], BF16)
        make_upper_triangular(nc, triu, 1.0, diag=True)
        ident_bf = consts.tile([128, 128], BF16)
        make_identity(nc, ident_bf)
        bq_sb = consts.tile([HC, 1], F32)
        nc.sync.dma_start(out=bq_sb, in_=bq[:, :])
        bk_sb = consts.tile([HC, 1], F32)
        nc.sync.dma_start(out=bk_sb, in_=bk[:, :])
        bv_sb = consts.tile([HC, 1], F32)
        nc.sync.dma_start(out=bv_sb, in_=bv[:, :])
        bo_sb = consts.tile([128, D], F32)
        nc.sync.dma_start(out=bo_sb, in_=_bcast(bo, 128, D))

        # --- small weights first (proj(0) needs them immediately) ---
        wq_sb = wpool.tile([128, DC, HC], BF16, tag="wq_sb")
        nc.sync.dma_start(out=wq_sb, in_=wq.rearrange("(c p) m -> p c m", p=128))
        wk_sb = wpool.tile([128, DC, HC], BF16, tag="wk_sb")
        nc.sync.dma_start(out=wk_sb, in_=wk.rearrange("(c p) m -> p c m", p=128))
        wv_sb = wpool.tile([128, DC, HC], BF16, tag="wv_sb")
        nc.sync.dma_start(out=wv_sb, in_=wv.rearrange("(c p) m -> p c m", p=128))

        def emit_xt_dma(b):
            xT = xtp.tile([128, DC, S], BF16, tag="xT", name=f"xT_{b}")
            for c in range(DC):
                nc.sync.dma_start(
                    out=xT[:, c, :],
                    in_=xt[c * 128:(c + 1) * 128, b * S:(b + 1) * S])
            return xT

        def proj_steps(b, xT):
            """Generator: K^T/Q^T/V^T projection + V DMA-transpose for batch
            b, yielded in PE-dense steps so attention(b-1) emission can
            interleave them."""
            kt = kqv.tile([HC, S], BF16, tag="kt", name=f"kt_{b}")
            qt_ = kqv.tile([HC, S], BF16, tag="qt", name=f"qt_{b}")
            vt = kqv.tile([HC, S], BF16, tag="vt", name=f"vt_{b}")
            for dst, w_sb, b_sb in ((kt, wk_sb, bk_sb), (qt_, wq_sb, bq_sb),
                                    (vt, wv_sb, bv_sb)):
                for s2 in range(S // 1024):
                    # one weight load per c serves both 512-chunks of the pair;
                    # yield every 2 c's (~4 MMs) so the interleaved attention
                    # strips never starve behind a long projection burst
                    pp = [ps_pp.tile([128, 512], F32, tag="pp",
                                     name=f"pp_{b}_{s2}_{u}")
                          for u in range(2)]
                    for c in range(DC):
                        for u in range(2):
                            s4 = 2 * s2 + u
                            nc.tensor.matmul(
                                pp[u], lhsT=w_sb[:, c, :],
                                rhs=xT[:, c, s4 * 512:(s4 + 1) * 512],
                                start=(c == 0), stop=(c == DC - 1))
                        if c % 2 == 1:
                            yield None
                    for u in range(2):
                        s4 = 2 * s2 + u
                        nc.vector.tensor_scalar_add(
                            dst[:, s4 * 512:(s4 + 1) * 512], pp[u], b_sb)
                    yield None
            # V natural via PE transposes (an XBAR dma_start_transpose would
            # be serialized by bass against every prior collective, blocking
            # the sync queue for tens of us), DVE-copied into the per-head
            # [V_h|1] layout
            vsb = kqv.tile([128, ST, H_LOC * 65], BF16, tag="vsb",
                           name=f"vsb_{b}")
            v4 = vsb.rearrange("p s (h o) -> p s h o", o=65)
            nc.vector.memset(v4[:, :, :, 64:65], 1.0)
            for st_ in range(ST):
                pt = ps_pp.tile([128, 512], F32, tag="pp",
                                name=f"pt_{b}_{st_}")[:, 0:64].bitcast(BF16)
                nc.tensor.transpose(pt,
                                    vt[:, st_ * 128:(st_ + 1) * 128], ident_bf)
                nc.vector.tensor_copy(
                    v4[:, st_, :, 0:64],
                    pt.rearrange("p (h o) -> p h o", o=64))
                if st_ % 4 == 3:
                    yield None
            yield None
            kqv_tiles[b] = (kt, qt_, vsb)

        def oproj_steps(b):
            """Generator: output projection for batch b's q-rows (after its
            AllToAll). DMAs ride the GpSimd queue, which already serializes
            behind the collective, so they never block the sync queue."""
            # aT loads wait on the AllToAll: ride the GpSimd queue (already
            # post-collective ordered) so they never block the sync queue
            aT = atp.tile([128, DC, 256], BF16, tag="aT", name=f"aT_{b}")
            for c in range(DC):
                nc.gpsimd.dma_start(out=aT[:, c, :],
                                    in_=out_b[b][c * 128:(c + 1) * 128, :])
            yield None
            for qt2 in range(2):
                pp = [ps_pp.tile([128, 512], F32, tag="pp",
                                 name=f"ppo_{b}_{qt2}_{nh}")
                      for nh in range(2)]
                for c in range(DC):
                    for nh in range(2):
                        nc.tensor.matmul(
                            pp[nh], lhsT=aT[:, c, qt2 * 128:qt2 * 128 + 128],
                            rhs=wo_sb[:, c, nh * 512:(nh + 1) * 512],
                            start=(c == 0), stop=(c == DC - 1))
                    if c % 2 == 1:
                        yield None
                for nh in range(2):
                    osb = opool.tile([128, 512], F32, tag="osb")
                    nc.vector.tensor_add(osb, pp[nh],
                                         bo_sb[:, nh * 512:(nh + 1) * 512])
                    nc.sync.dma_start(
                        out=out[b * 256 + qt2 * 128:b * 256 + qt2 * 128 + 128,
                                nh * 512:(nh + 1) * 512],
                        in_=osb)
                yield None

        def _paced(gen, credit_per_yield):
            """Wrap a generator so each next() only advances it
            `credit_per_yield` steps on average — spreads interleaved work
            evenly across the attention strips instead of front-loading."""
            credit = 0.0
            while True:
                credit += credit_per_yield
                while credit >= 1.0:
                    credit -= 1.0
                    try:
                        next(gen)
                    except StopIteration:
                        return
                yield None

        def emit_attention(b, interleave):
            kt, qt_, vsb = kqv_tiles[b]
            for t in range(NT):
                q0 = 512 * t
                nj = 4 * t + 4
                po = [ps_po.tile([65, 512], F32, tag="po",
                                 name=f"po_{b}_{t}_{h}") for h in range(2)]

                def emit_pv(pend, po=po, nj=nj):
                    es_p, jp, relp = pend
                    for h in range(H_LOC):
                        nc.tensor.matmul(
                            po[h][:, relp:512],
                            lhsT=vsb[:, jp, h * 65:(h + 1) * 65],
                            rhs=es_p[:, h * 512 + relp:h * 512 + 512],
                            start=(jp == 0), stop=(jp == nj - 1),
                            skip_group_check=True)

                prev = None
                for j in range(nj):
                    rel = max(0, 128 * j - q0)
                    ps = ps_s.tile([128, 1024], F32, tag="ps")
                    # both heads' scores concurrently via PE row-group tiles
                    nc.tensor.matmul(ps[:, rel:512],
                                     lhsT=kt[0:64, j * 128:(j + 1) * 128],
                                     rhs=qt_[0:64, q0 + rel:q0 + 512],
                                     start=True, stop=True)
                    nc.tensor.matmul(ps[:, 512 + rel:1024],
                                     lhsT=kt[64:128, j * 128:(j + 1) * 128],
                                     rhs=qt_[64:128, q0 + rel:q0 + 512],
                                     start=True, stop=True,
                                     skip_group_check=True)
                    es = epool.tile([128, 1024], BF16, tag="et")
                    # one wide exp covers both heads ([512:512+rel) is unused
                    # garbage on diagonal strips, never consumed by PV)
                    nc.scalar.activation(es[:, rel:1024], ps[:, rel:1024],
                                         mybir.ActivationFunctionType.Exp,
                                         scale=1.0 / np.sqrt(DK))
                    if 128 * j >= q0:  # diagonal strip: causal 0/1 mask
                        nc.vector.tensor_mul(es[:, rel:rel + 128],
                                             es[:, rel:rel + 128], triu)
                        nc.vector.tensor_mul(es[:, 512 + rel:512 + rel + 128],
                                             es[:, 512 + rel:512 + rel + 128],
                                             triu)
                    if prev:
                        emit_pv(prev)
                        next(interleave, None)
                    prev = (es, j, rel)
                    next(interleave, None)
                emit_pv(prev)
                next(interleave, None)

                # normalize (no transposes: po is already [head-dim, q]).
                # DVE copy to SBUF releases the po PSUM banks immediately.
                # The denominator row lives on ONE partition, where a DVE
                # reciprocal would be serial (8 cyc/elem = 3.4us); instead
                # DMA-spread it across 64 lanes, reciprocal there (0.1us),
                # gather back and broadcast-read. The whole chain rides the
                # GpSimd DMA queue so it never blocks sync-queue traffic.
                poc = rpool.tile([65, 1024], F32, tag="poc")
                nc.vector.tensor_copy(poc[:, 0:512], po[0])
                nc.vector.tensor_copy(poc[:, 512:1024], po[1])
                slot = (4 * b + t) * 2048
                nc.gpsimd.dma_start(out=rcpd[0:1, slot:slot + 1024],
                                    in_=poc[64:65, :])
                dsc = spool.tile([64, 16], F32, tag="dsc")
                nc.gpsimd.dma_start(
                    out=dsc,
                    in_=rcpd[0:1, slot:slot + 1024].rearrange(
                        "a (b c) -> (a b) c", c=16))
                rcv = spool.tile([64, 16], F32, tag="rcv")
                nc.vector.reciprocal(rcv, dsc)
                nc.gpsimd.dma_start(
                    out=rcpd[0:1, slot + 1024:slot + 2048].rearrange(
                        "a (b c) -> (a b) c", c=16),
                    in_=rcv)
                rb = bpool.tile([64, 1024], F32, tag="rb")
                nc.gpsimd.dma_start(
                    out=rb,
                    in_=rcpd[0:1, slot + 1024:slot + 2048].to_broadcast(
                        [64, 1024]))
                for h in range(H_LOC):
                    # normalize-mul runs on GpSimd: it serializes after its
                    # rb load on the same queue, so a late rb never blocks
                    # the DVE queue (whose mask-muls gate the PE pipeline)
                    st = spool.tile([64, 512], BF16, tag="st")
                    nc.gpsimd.tensor_mul(st, poc[0:64, h * 512:(h + 1) * 512],
                                         rb[0:64, h * 512:(h + 1) * 512])
                    for u in range(2):
                        s = 2 * t + u
                        nc.gpsimd.dma_start(
                            out=in_b[b][s * 128 + h * 64:s * 128 + h * 64 + 64,
                                        0:256],
                            in_=st[:, u * 256:(u + 1) * 256])
                next(interleave, None)

            nc.gpsimd.collective_compute(
                "AllToAll", mybir.AluOpType.bypass,
                replica_groups=[list(range(N_CORES))],
                ins=[in_b[b].opt()], outs=[out_b[b].opt()])

        # warm up the collective path while the pipeline head loads/projects
        warm_sb = consts.tile([N_CORES, 16], BF16)
        nc.sync.dma_start(out=warm_sb, in_=xt[0:N_CORES, 0:16])
        nc.gpsimd.dma_start(out=warm_i[:, :], in_=warm_sb)
        nc.gpsimd.collective_compute(
            "AllToAll", mybir.AluOpType.bypass,
            replica_groups=[list(range(N_CORES))],
            ins=[warm_i.opt()], outs=[warm_o.opt()])

        kqv_tiles = {}
        xT0 = emit_xt_dma(0)
        # wo is only needed by oproj(0), well after proj(0): load it after
        # xT(0) so it doesn't delay the pipeline head on the sync DMA queue
        wo_sb = wpool.tile([128, DC, D], BF16, tag="wo_sb")
        nc.sync.dma_start(out=wo_sb, in_=wo.rearrange("(c p) m -> p c m", p=128))

        for _ in proj_steps(0, xT0):
            pass
        pending = []
        for b in range(B):
            gens = []
            if b + 1 < B:
                xTn = emit_xt_dma(b + 1)
                gens.append(proj_steps(b + 1, xTn))
            gens.extend(pending)
            pending = []
            # ~42 interleave steps across ~84 attention yield points; run
            # slightly ahead so proj(b+1) is done before attention(b+1)
            inter = _paced(chain(*gens), 1.0) if gens else iter(())
            emit_attention(b, inter)
            for _ in inter:
                pass
            pending = [oproj_steps(b)]
        for g in pending:
            for _ in g:
                pass

    nc.compile()
    return nc


_NC_CACHE = None


def _get_program():
    global _NC_CACHE
    if _NC_CACHE is None:
        _NC_CACHE = build_program()
    return _NC_CACHE


def _make_in_maps(x, w_qkv, b_qkv, w_o, b_o):
    x = np.asarray(x, dtype=np.float32).reshape(B * S, D)
    xt = np.ascontiguousarray(x.T).astype(BF16_NP)
    w_qkv = np.asarray(w_qkv, dtype=np.float32)
    b_qkv = np.asarray(b_qkv, dtype=np.float32)
    wo_bf = np.ascontiguousarray(np.asarray(w_o, dtype=np.float32)).astype(BF16_NP)
    b_o = np.asarray(b_o, dtype=np.float32).reshape(1, D)
    in_maps = []
    for c in range(N_CORES):
        lo = c * HC
        hi = lo + HC
        in_maps.append({
            "xt": xt,
            "wq": np.ascontiguousarray(w_qkv[:, lo:hi]).astype(BF16_NP),
            "wk": np.ascontiguousarray(w_qkv[:, D + lo:D + hi]).astype(BF16_NP),
            "wv": np.ascontiguousarray(w_qkv[:, 2 * D + lo:2 * D + hi]).astype(BF16_NP),
            "bq": np.ascontiguousarray(b_qkv[lo:hi].reshape(HC, 1)),
            "bk": np.ascontiguousarray(b_qkv[D + lo:D + hi].reshape(HC, 1)),
            "bv": np.ascontiguousarray(b_qkv[2 * D + lo:2 * D + hi].reshape(HC, 1)),
            "wo": wo_bf,
            "bo": b_o,
        })
    return in_maps


def _assemble(results):
    out = np.empty((B, S, D), dtype=np.float32)
    for c in range(N_CORES):
        q0 = c * 256
        for k in range(B):
            out[k, q0:q0 + 256, :] = results[c]["out"][k * 256:(k + 1) * 256]
    return out


def run(x, mask, w_qkv, b_qkv, w_o, b_o, trace=False, **trace_kwargs):
    """Run on hardware; returns (output, BassKernelResults)."""
    nc = _get_program()
    in_maps = _make_in_maps(x, w_qkv, b_qkv, w_o, b_o)
    res = run_bass_kernel_spmd(nc, in_maps, list(range(N_CORES)),
                               trace=trace, **trace_kwargs)
    return _assemble(res.results), res


def kernel(x, mask, w_qkv, b_qkv, w_o, b_o):
    out, _ = run(x, mask, w_qkv, b_qkv, w_o, b_o)
    return out


# revision 33
# speedup vs baseline: 1.1222x; 1.0200x over previous
"""MultiHeadAttention (B=4, S=2048, D=1024, H=16, causal) on 8 TRN2 NeuronCores.

Sharding: tensor-parallel over heads across all 8 cores (2 heads/core, all 4
batches processed locally; identical SPMD control flow on every core). After
attention, an 8-core AllToAll per batch redistributes attention outputs so
each core runs the output projection for 1/8 of the (batch, seq) rows.

Per-core pipeline (all matmuls bf16 with f32 PSUM accumulation):
  - x arrives host-transposed as x^T [D, B*S] bf16; K^T/Q^T/V^T via
    w-stationary matmuls (each weight load serves a pair of 512-wide chunks),
    bias added on the DVE eviction. V^T is transposed to natural V by the DMA
    XBAR engine (no PE involvement) with a ones column per head so the PV
    matmul also produces the softmax denominator.
  - Scores are computed transposed ([k, q] = K @ Q^T) in 512-wide q-chunks.
    The two heads' score matmuls are issued as row-group tiles (K=64 each,
    rows 0-63 / 64-127) so they execute CONCURRENTLY in the PE array and land
    in the two PSUM banks of one [128, 1024] tile; a single wide exp on
    ScalarE covers both heads (1/sqrt(dk) scale folded in; no max subtraction
    needed: |scores| <~ 2.6). Causal mask = 0/1 triangular multiply on
    diagonal tiles only; fully-masked tiles are never computed.
  - PV: [V_h|1] stationary, exp chunks stream, accumulating po_h = [out^T;
    denom] [65, 512] in PSUM, one k-strip behind scores/exp so the in-order
    PE queue never head-of-line blocks on ScalarE.
  - po is already in the [head-dim, q] layout the AllToAll needs, so there
    are no output transposes at all: the denominator row is reciprocal'd,
    bounced through DRAM to broadcast it across 64 partitions, and a single
    DVE multiply writes the normalized bf16 attention output.
  - Output projection for batch b runs right after its AllToAll, interleaved
    into the next batch's attention (its DMAs ride the GpSimd queue, which
    already serializes behind the collective).
"""

import sys

if "/opt/trn_rl_repo" not in sys.path:
    sys.path.insert(0, "/opt/trn_rl_repo")

from contextlib import ExitStack
from itertools import chain

import ml_dtypes
import numpy as np

import concourse.bacc as bacc
import concourse.bass as bass
import concourse.mybir as mybir
import concourse.tile as tile
from concourse.bass_utils import run_bass_kernel_spmd
from concourse.masks import make_identity, make_upper_triangular

N_CORES = 8
B = 4
S = 2048
D = 1024
H_TOT = 16
DK = 64
H_LOC = H_TOT // N_CORES  # 2 heads per core
HC = H_LOC * DK  # 128 head-cols per core
ST = S // 128  # 16 k-strips per batch
DC = D // 128  # 8 d_model chunks
NT = S // 512  # 4 q-chunks per batch
BQ = (B * S) // N_CORES  # 1024 (batch,seq) rows per core after AllToAll

F32 = mybir.dt.float32
BF16 = mybir.dt.bfloat16
BF16_NP = ml_dtypes.bfloat16


def _bcast(handle, rows, cols):
    """AP reading a [1, cols] DRAM tensor broadcast over `rows` partitions."""
    return bass.AP(tensor=handle, offset=0, ap=[[0, rows], [1, cols]])


def build_program():
    nc = bacc.Bacc("TRN2", target_bir_lowering=False, debug=False,
                   num_devices=N_CORES)

    xt = nc.declare_dram_parameter("xt", [D, B * S], BF16, isOutput=False)
    wq = nc.declare_dram_parameter("wq", [D, HC], BF16, isOutput=False)
    wk = nc.declare_dram_parameter("wk", [D, HC], BF16, isOutput=False)
    wv = nc.declare_dram_parameter("wv", [D, HC], BF16, isOutput=False)
    bq = nc.declare_dram_parameter("bq", [HC, 1], F32, isOutput=False)
    bk = nc.declare_dram_parameter("bk", [HC, 1], F32, isOutput=False)
    bv = nc.declare_dram_parameter("bv", [HC, 1], F32, isOutput=False)
    wo = nc.declare_dram_parameter("wo", [D, D], BF16, isOutput=False)
    bo = nc.declare_dram_parameter("bo", [1, D], F32, isOutput=False)
    out = nc.declare_dram_parameter("out", [BQ, D], F32, isOutput=True)

    with ExitStack() as ctx:
        tc = ctx.enter_context(tile.TileContext(nc))

        consts = ctx.enter_context(tc.tile_pool(name="consts", bufs=1))
        wpool = ctx.enter_context(tc.tile_pool(name="wpool", bufs=1))
        xtp = ctx.enter_context(tc.tile_pool(name="xtp", bufs=2))
        kqv = ctx.enter_context(tc.tile_pool(name="kqv", bufs=2))
        epool = ctx.enter_context(tc.tile_pool(name="epool", bufs=4))
        # normalize-chain pools are deep enough (4 chunks = one full batch)
        # to ride out the first-collective setup (~56us) blocking the GpSimd
        # DMA queue without backpressuring the PV/exp pipeline
        rpool = ctx.enter_context(tc.tile_pool(name="rpool", bufs=5))
        bpool = ctx.enter_context(tc.tile_pool(name="bpool", bufs=5))
        spool = ctx.enter_context(tc.tile_pool(name="spool", bufs=10))
        opool = ctx.enter_context(tc.tile_pool(name="opool", bufs=2))
        atp = ctx.enter_context(tc.tile_pool(name="atp", bufs=2))
        ps_s = ctx.enter_context(tc.tile_pool(name="ps_s", bufs=2, space="PSUM"))
        ps_po = ctx.enter_context(tc.tile_pool(name="ps_po", bufs=2, space="PSUM"))
        ps_pp = ctx.enter_context(tc.tile_pool(name="ps_pp", bufs=2, space="PSUM"))
        dram = ctx.enter_context(tc.tile_pool(name="dram", bufs=1, space="DRAM"))

        # tiny dummy exchange issued at kernel start: absorbs the one-time
        # first-collective setup (~40us) concurrently with the initial
        # DMA/projection phase instead of exposing it on batch 0's critical
        # path
        warm_i = dram.tile([N_CORES, 16], BF16, tag="warm_i", name="warm_i")
        warm_o = dram.tile([N_CORES, 16], BF16, tag="warm_o", name="warm_o")
        # four quarter-exchanges (one per batch)
        in_b = [dram.tile([N_CORES * 128, 256], BF16, tag=f"in_b{i}",
                          name=f"in_b{i}") for i in range(B)]
        out_b = [dram.tile([N_CORES * 128, 256], BF16, tag=f"out_b{i}",
                           name=f"out_b{i}") for i in range(B)]
        # DRAM bounce for the reciprocal-denominator partition broadcast
        rcpd = dram.tile([1, B * NT * 2048], F32, tag="rcpd", name="rcpd")

        # --- constants ---
        triu = consts.tile([128, 128], BF16)
        make_upper_triangular(nc, triu, 1.0, diag=True)
        ident_bf = consts.tile([128, 128], BF16)
        make_identity(nc, ident_bf)
        bq_sb = consts.tile([HC, 1], F32)
        nc.sync.dma_start(out=bq_sb, in_=bq[:, :])
        bk_sb = consts.tile([HC, 1], F32)
        nc.sync.dma_start(out=bk_sb, in_=bk[:, :])
        bv_sb = consts.tile([HC, 1], F32)
        nc.sync.dma_start(out=bv_sb, in_=bv[:, :])
        bo_sb = consts.tile([128, D], F32)
        nc.sync.dma_start(out=bo_sb, in_=_bcast(bo, 128, D))

        # --- small weights first (proj(0) needs them immediately) ---
        wq_sb = wpool.tile([128, DC, HC], BF16, tag="wq_sb")
        nc.sync.dma_start(out=wq_sb, in_=wq.rearrange("(c p) m -> p c m", p=128))
        wk_sb = wpool.tile([128, DC, HC], BF16, tag="wk_sb")
        nc.sync.dma_start(out=wk_sb, in_=wk.rearrange("(c p) m -> p c m", p=128))
        wv_sb = wpool.tile([128, DC, HC], BF16, tag="wv_sb")
        nc.sync.dma_start(out=wv_sb, in_=wv.rearrange("(c p) m -> p c m", p=128))

        def emit_xt_dma(b):
            xT = xtp.tile([128, DC, S], BF16, tag="xT", name=f"xT_{b}")
            for c in range(DC):
                nc.sync.dma_start(
                    out=xT[:, c, :],
                    in_=xt[c * 128:(c + 1) * 128, b * S:(b + 1) * S])
            return xT

        def proj_steps(b, xT):
            """Generator: K^T/Q^T/V^T projection + V DMA-transpose for batch
            b, yielded in PE-dense steps so attention(b-1) emission can
            interleave them."""
            kt = kqv.tile([HC, S], BF16, tag="kt", name=f"kt_{b}")
            qt_ = kqv.tile([HC, S], BF16, tag="qt", name=f"qt_{b}")
            vt = kqv.tile([HC, S], BF16, tag="vt", name=f"vt_{b}")
            for dst, w_sb, b_sb in ((kt, wk_sb, bk_sb), (qt_, wq_sb, bq_sb),
                                    (vt, wv_sb, bv_sb)):
                for s2 in range(S // 1024):
                    # one weight load per c serves both 512-chunks of the pair;
                    # yield every 2 c's (~4 MMs) so the interleaved attention
                    # strips never starve behind a long projection burst
                    pp = [ps_pp.tile([128, 512], F32, tag="pp",
                                     name=f"pp_{b}_{s2}_{u}")
                          for u in range(2)]
                    for c in range(DC):
                        for u in range(2):
                            s4 = 2 * s2 + u
                            nc.tensor.matmul(
                                pp[u], lhsT=w_sb[:, c, :],
                                rhs=xT[:, c, s4 * 512:(s4 + 1) * 512],
                                start=(c == 0), stop=(c == DC - 1))
                        if c % 2 == 1:
                            yield None
                    for u in range(2):
                        s4 = 2 * s2 + u
                        nc.vector.tensor_scalar_add(
                            dst[:, s4 * 512:(s4 + 1) * 512], pp[u], b_sb)
                    yield None
            # V natural via PE transposes (an XBAR dma_start_transpose would
            # be serialized by bass against every prior collective, blocking
            # the sync queue for tens of us), DVE-copied into the per-head
            # [V_h|1] layout
            vsb = kqv.tile([128, ST, H_LOC * 65], BF16, tag="vsb",
                           name=f"vsb_{b}")
            v4 = vsb.rearrange("p s (h o) -> p s h o", o=65)
            nc.vector.memset(v4[:, :, :, 64:65], 1.0)
            for st_ in range(ST):
                pt = ps_pp.tile([128, 512], F32, tag="pp",
                                name=f"pt_{b}_{st_}")[:, 0:64].bitcast(BF16)
                nc.tensor.transpose(pt,
                                    vt[:, st_ * 128:(st_ + 1) * 128], ident_bf)
                nc.vector.tensor_copy(
                    v4[:, st_, :, 0:64],
                    pt.rearrange("p (h o) -> p h o", o=64))
                if st_ % 4 == 3:
                    yield None
            yield None
            kqv_tiles[b] = (kt, qt_, vsb)

        def oproj_steps(b):
            """Generator: output projection for batch b's q-rows (after its
            AllToAll). DMAs ride the GpSimd queue, which already serializes
            behind the collective, so they never block the sync queue."""
            # aT loads wait on the AllToAll: ride the GpSimd queue (already
            # post-collective ordered) so they never block the sync queue
            aT = atp.tile([128, DC, 256], BF16, tag="aT", name=f"aT_{b}")
            for c in range(DC):
                nc.gpsimd.dma_start(out=aT[:, c, :],
                                    in_=out_b[b][c * 128:(c + 1) * 128, :])
            yield None
            for qt2 in range(2):
                pp = [ps_pp.tile([128, 512], F32, tag="pp",
                                 name=f"ppo_{b}_{qt2}_{nh}")
                      for nh in range(2)]
                for c in range(DC):
                    for nh in range(2):
                        nc.tensor.matmul(
                            pp[nh], lhsT=aT[:, c, qt2 * 128:qt2 * 128 + 128],
                            rhs=wo_sb[:, c, nh * 512:(nh + 1) * 512],
                            start=(c == 0), stop=(c == DC - 1))
                    if c % 2 == 1:
                        yield None
                for nh in range(2):
                    osb = opool.tile([128, 512], F32, tag="osb")
                    nc.vector.tensor_add(osb, pp[nh],
                                         bo_sb[:, nh * 512:(nh + 1) * 512])
                    nc.sync.dma_start(
                        out=out[b * 256 + qt2 * 128:b * 256 + qt2 * 128 + 128,
                                nh * 512:(nh + 1) * 512],
                        in_=osb)
                yield None

        def _paced(gen, credit_per_yield):
            """Wrap a generator so each next() only advances it
            `credit_per_yield` steps on average — spreads interleaved work
            evenly across the attention strips instead of front-loading."""
            credit = 0.0
            while True:
                credit += credit_per_yield
                while credit >= 1.0:
                    credit -= 1.0
                    try:
                        next(gen)
                    except StopIteration:
                        return
                yield None

        def emit_attention(b, interleave):
            kt, qt_, vsb = kqv_tiles[b]
            for t in range(NT):
                q0 = 512 * t
                nj = 4 * t + 4
                po = [ps_po.tile([65, 512], F32, tag="po",
                                 name=f"po_{b}_{t}_{h}") for h in range(2)]

                def emit_pv(pend, po=po, nj=nj):
                    es_p, jp, relp = pend
                    for h in range(H_LOC):
                        nc.tensor.matmul(
                            po[h][:, relp:512],
                            lhsT=vsb[:, jp, h * 65:(h + 1) * 65],
                            rhs=es_p[:, h * 512 + relp:h * 512 + 512],
                            start=(jp == 0), stop=(jp == nj - 1),
                            skip_group_check=True)

                prev = None
                for j in range(nj):
                    rel = max(0, 128 * j - q0)
                    ps = ps_s.tile([128, 1024], F32, tag="ps")
                    # both heads' scores concurrently via PE row-group tiles
                    nc.tensor.matmul(ps[:, rel:512],
                                     lhsT=kt[0:64, j * 128:(j + 1) * 128],
                                     rhs=qt_[0:64, q0 + rel:q0 + 512],
                                     start=True, stop=True)
                    nc.tensor.matmul(ps[:, 512 + rel:1024],
                                     lhsT=kt[64:128, j * 128:(j + 1) * 128],
                                     rhs=qt_[64:128, q0 + rel:q0 + 512],
                                     start=True, stop=True,
                                     skip_group_check=True)
                    es = epool.tile([128, 1024], BF16, tag="et")
                    # one wide exp covers both heads ([512:512+rel) is unused
                    # garbage on diagonal strips, never consumed by PV)
                    nc.scalar.activation(es[:, rel:1024], ps[:, rel:1024],
                                         mybir.ActivationFunctionType.Exp,
                                         scale=1.0 / np.sqrt(DK))
                    if 128 * j >= q0:  # diagonal strip: causal 0/1 mask
                        nc.vector.tensor_mul(es[:, rel:rel + 128],
                                             es[:, rel:rel + 128], triu)
                        nc.vector.tensor_mul(es[:, 512 + rel:512 + rel + 128],
                                             es[:, 512 + rel:512 + rel + 128],
                                             triu)
                    if prev:
                        emit_pv(prev)
                        next(interleave, None)
                    prev = (es, j, rel)
                    next(interleave, None)
                emit_pv(prev)
                next(interleave, None)

                # normalize (no transposes: po is already [head-dim, q]).
                # DVE copy to SBUF releases the po PSUM banks immediately.
                # The denominator row lives on ONE partition, where a DVE
                # reciprocal would be serial (8 cyc/elem = 3.4us); instead
                # DMA-spread it across 64 lanes, reciprocal there (0.1us),
                # gather back and broadcast-read. The whole chain rides the
                # GpSimd DMA queue so it never blocks sync-queue traffic.
                # batch 0's chain rides the sync queue (the warmup collective
                # blocks the GpSimd queue ~50us at the start); later batches
                # ride GpSimd so collective-adjacent DMAs stay off sync
                dq = nc.sync if b == 0 else nc.gpsimd
                poc = rpool.tile([65, 1024], F32, tag="poc")
                nc.vector.tensor_copy(poc[:, 0:512], po[0])
                nc.vector.tensor_copy(poc[:, 512:1024], po[1])
                slot = (4 * b + t) * 2048
                dq.dma_start(out=rcpd[0:1, slot:slot + 1024],
                             in_=poc[64:65, :])
                dsc = spool.tile([64, 16], F32, tag="dsc")
                dq.dma_start(
                    out=dsc,
                    in_=rcpd[0:1, slot:slot + 1024].rearrange(
                        "a (b c) -> (a b) c", c=16))
                rcv = spool.tile([64, 16], F32, tag="rcv")
                nc.vector.reciprocal(rcv, dsc)
                dq.dma_start(
                    out=rcpd[0:1, slot + 1024:slot + 2048].rearrange(
                        "a (b c) -> (a b) c", c=16),
                    in_=rcv)
                rb = bpool.tile([64, 1024], F32, tag="rb")
                dq.dma_start(
                    out=rb,
                    in_=rcpd[0:1, slot + 1024:slot + 2048].to_broadcast(
                        [64, 1024]))
                for h in range(H_LOC):
                    st = spool.tile([64, 512], BF16, tag="st")
                    nc.vector.tensor_mul(st, poc[0:64, h * 512:(h + 1) * 512],
                                         rb[0:64, h * 512:(h + 1) * 512])
                    for u in range(2):
                        s = 2 * t + u
                        dq.dma_start(
                            out=in_b[b][s * 128 + h * 64:s * 128 + h * 64 + 64,
                                        0:256],
                            in_=st[:, u * 256:(u + 1) * 256])
                next(interleave, None)

            nc.gpsimd.collective_compute(
                "AllToAll", mybir.AluOpType.bypass,
                replica_groups=[list(range(N_CORES))],
                ins=[in_b[b].opt()], outs=[out_b[b].opt()])

        # warm up the collective path while the pipeline head loads/projects
        warm_sb = consts.tile([N_CORES, 16], BF16)
        nc.sync.dma_start(out=warm_sb, in_=xt[0:N_CORES, 0:16])
        nc.gpsimd.dma_start(out=warm_i[:, :], in_=warm_sb)
        nc.gpsimd.collective_compute(
            "AllToAll", mybir.AluOpType.bypass,
            replica_groups=[list(range(N_CORES))],
            ins=[warm_i.opt()], outs=[warm_o.opt()])

        kqv_tiles = {}
        xT0 = emit_xt_dma(0)
        # wo is only needed by oproj(0), well after proj(0): load it after
        # xT(0) so it doesn't delay the pipeline head on the sync DMA queue
        wo_sb = wpool.tile([128, DC, D], BF16, tag="wo_sb")
        nc.sync.dma_start(out=wo_sb, in_=wo.rearrange("(c p) m -> p c m", p=128))

        for _ in proj_steps(0, xT0):
            pass
        pending = []
        for b in range(B):
            gens = []
            if b + 1 < B:
                xTn = emit_xt_dma(b + 1)
                gens.append(proj_steps(b + 1, xTn))
            if b < B - 1:
                # oproj(2) is NOT interleaved into attention(3): deferring it
                # to the end lets its matmuls overlap the final AllToAll
                gens.extend(pending)
                pending = []
            # ~42 interleave steps across ~84 attention yield points; run
            # slightly ahead so proj(b+1) is done before attention(b+1)
            inter = _paced(chain(*gens), 1.0) if gens else iter(())
            emit_attention(b, inter)
            for _ in inter:
                pass
            pending.append(oproj_steps(b))
        for g in pending:
            for _ in g:
                pass

    nc.compile()
    return nc


_NC_CACHE = None


def _get_program():
    global _NC_CACHE
    if _NC_CACHE is None:
        _NC_CACHE = build_program()
    return _NC_CACHE


def _make_in_maps(x, w_qkv, b_qkv, w_o, b_o):
    x = np.asarray(x, dtype=np.float32).reshape(B * S, D)
    xt = np.ascontiguousarray(x.T).astype(BF16_NP)
    w_qkv = np.asarray(w_qkv, dtype=np.float32)
    b_qkv = np.asarray(b_qkv, dtype=np.float32)
    wo_bf = np.ascontiguousarray(np.asarray(w_o, dtype=np.float32)).astype(BF16_NP)
    b_o = np.asarray(b_o, dtype=np.float32).reshape(1, D)
    in_maps = []
    for c in range(N_CORES):
        lo = c * HC
        hi = lo + HC
        in_maps.append({
            "xt": xt,
            "wq": np.ascontiguousarray(w_qkv[:, lo:hi]).astype(BF16_NP),
            "wk": np.ascontiguousarray(w_qkv[:, D + lo:D + hi]).astype(BF16_NP),
            "wv": np.ascontiguousarray(w_qkv[:, 2 * D + lo:2 * D + hi]).astype(BF16_NP),
            "bq": np.ascontiguousarray(b_qkv[lo:hi].reshape(HC, 1)),
            "bk": np.ascontiguousarray(b_qkv[D + lo:D + hi].reshape(HC, 1)),
            "bv": np.ascontiguousarray(b_qkv[2 * D + lo:2 * D + hi].reshape(HC, 1)),
            "wo": wo_bf,
            "bo": b_o,
        })
    return in_maps


def _assemble(results):
    out = np.empty((B, S, D), dtype=np.float32)
    for c in range(N_CORES):
        q0 = c * 256
        for k in range(B):
            out[k, q0:q0 + 256, :] = results[c]["out"][k * 256:(k + 1) * 256]
    return out


def run(x, mask, w_qkv, b_qkv, w_o, b_o, trace=False, **trace_kwargs):
    """Run on hardware; returns (output, BassKernelResults)."""
    nc = _get_program()
    in_maps = _make_in_maps(x, w_qkv, b_qkv, w_o, b_o)
    res = run_bass_kernel_spmd(nc, in_maps, list(range(N_CORES)),
                               trace=trace, **trace_kwargs)
    return _assemble(res.results), res


def kernel(x, mask, w_qkv, b_qkv, w_o, b_o):
    out, _ = run(x, mask, w_qkv, b_qkv, w_o, b_o)
    return out
